# revision 15
# baseline (speedup 1.0000x reference)
"""Distributed Trainium2 (8 NeuronCore) multi-head attention kernel, v2.

Problem: y = softmax((x Wq)(x Wk)^T * DIM**-0.5) (x Wv) Wo + bo
  x: [4096, 256], 8 heads of dim 32, scale by full-dim**-0.5 (1/16).

Sharding: head-parallel, one head per core. v2 restructures the matmul
orientations so the PE never loads large stationary operands:

  - QKV projection with W stationary; q^T/k^T are produced REPLICATED in
    four 32-partition bands via PE column tiling (4 concurrent matmuls
    share the x stream), so scores can use PE row tiling.
  - Scores S^T[j, i-block]: kT stationary [32,128] per j-tile, qT moving
    512 wide; two j-tiles run concurrently in PE row bands {0,32}.
  - exp: ScalarE spline (10/16 slots) + DVE Schraudolph bf16 bit-trick
    (6/16 slots), both reading f32 PSUM score slots.
  - AV: V stationary (tiny, 32 cols) streaming P^T 512 wide; four
    concurrent col-tiled matmuls per j-pair produce O_even^T/O_odd^T in
    PSUM partition bands 0-31/32-63 plus softmax row-sums in bands 64/96
    via ones-column stationaries. The halves are summed after a DMA-xbar
    transpose into [i, d] orientation where the softmax division is a
    cheap per-partition reciprocal + scale.
  - Output exchange: one AllToAll of O^T (256KB bf16 per core), then the
    projection with Wout stationary and O^T moving produces y^T; the host
    transposes each core's [256, 512] slice back to rows.
"""

import numpy as np

P = 128          # partitions
N = 4096         # sequence length
DIM = 256        # model dim
H = 8            # heads == cores
D = DIM // H     # head dim, 32
NCORES = 8
W = 512          # i-columns per pass
NPASS = N // W   # 8 passes; pass p covers dest core p's rows
NT = N // P      # 32 j-tiles
NSLOT = NT // 2  # 16 slots of 2 j-tiles per pass
RPC = N // NCORES
SCALE = DIM ** -0.5

# Schraudolph bf16 fast-exp: bits(exp(s*SCALE)) ~= s*FE_A + FE_B (int16)
FE_A = 128.0 * SCALE * 1.4426950408889634
FE_B = 16256.0 - 4.6

# Per-slot exp engine: True = DVE Schraudolph, False = ScalarE spline.
EXP_DVE = (False, False, True, False, True, False, True, False, True,
           False, True, False, True, False, True, False)
FILL_N = 2       # HAM-warming filler matmuls per slot
assert len(EXP_DVE) == NSLOT
# row index of each slot's 2 j-rows inside its engine's P^T tile
_sc_rows, _dv_rows = {}, {}
for _s, _d in enumerate(EXP_DVE):
    if _d:
        _dv_rows[_s] = 2 * len(_dv_rows)
    else:
        _sc_rows[_s] = 2 * len(_sc_rows)
N_SC_J = 2 * len(_sc_rows)   # 20
N_DV_J = 2 * len(_dv_rows)   # 12

_BUILT = None


def _build():
    from contextlib import ExitStack

    import concourse.mybir as mybir
    import concourse.tile as tile
    from concourse import bacc

    f32 = mybir.dt.float32
    bf16 = mybir.dt.bfloat16
    i16 = mybir.dt.int16
    AF = mybir.ActivationFunctionType
    ALU = mybir.AluOpType

    nc = bacc.Bacc("TRN2", target_bir_lowering=False, debug=False,
                   num_devices=NCORES)
    xb = nc.dram_tensor("xb", [DIM, N], bf16, kind="ExternalInput")
    wqkv = nc.dram_tensor("wqkv", [DIM, 3 * D], bf16, kind="ExternalInput")
    bqkv4 = nc.dram_tensor("bqkv4", [P, 3], f32, kind="ExternalInput")
    wo = nc.dram_tensor("wo", [DIM, DIM], bf16, kind="ExternalInput")
    bo = nc.dram_tensor("bo", [1, DIM], bf16, kind="ExternalInput")
    outT = nc.dram_tensor("outT", [DIM, RPC], f32, kind="ExternalOutput")

    with tile.TileContext(nc) as tc, ExitStack() as ctx:
        singles = ctx.enter_context(tc.tile_pool(name="singles", bufs=1))
        sm_pool = ctx.enter_context(tc.tile_pool(name="sm", bufs=3))
        pt_pool = ctx.enter_context(tc.tile_pool(name="ptp", bufs=2))
        st_pool = ctx.enter_context(
            tc.tile_pool(name="stp", bufs=2, space="PSUM"))
        acc_pool = ctx.enter_context(
            tc.tile_pool(name="accp", bufs=2, space="PSUM"))
        prj_pool = ctx.enter_context(
            tc.tile_pool(name="prjp", bufs=1, space="PSUM"))
        dram = ctx.enter_context(
            tc.tile_pool(name="dram", bufs=1, space="DRAM"))

        # ---------------- input loads ----------------
        xs = singles.tile([P, 2, N], bf16)
        for c in range(2):
            for t in range(4):
                sl = slice(t * (N // 4), (t + 1) * (N // 4))
                nc.sync.dma_start(out=xs[:, c, sl], in_=xb[c * P:(c + 1) * P, sl])

        ws = singles.tile([P, 2, 3 * D], bf16)
        for c in range(2):
            nc.sync.dma_start(out=ws[:, c, :], in_=wqkv[c * P:(c + 1) * P, :])
        bq4 = singles.tile([P, 3], f32)
        nc.sync.dma_start(out=bq4[:], in_=bqkv4[:, :])
        wos = singles.tile([P, 2, DIM], bf16)
        for c in range(2):
            nc.sync.dma_start(out=wos[:, c, :], in_=wo[c * P:(c + 1) * P, :])
        bos = singles.tile([1, DIM], bf16)
        nc.sync.dma_start(out=bos[:], in_=bo[:, :])

        ones1 = singles.tile([P, 1], bf16)
        nc.vector.memset(ones1[:], 1.0)
        onesw = singles.tile([1, W], bf16)
        nc.vector.memset(onesw[:], 1.0)


        # ------- QKV projection -------
        # qT4/kT4: [128, 4096] bf16, four replicated 32-row bands
        # (col-tiled matmuls share the x stream). vTs: [32, 4096] bf16.
        qT4 = singles.tile([P, N], bf16)
        kT4 = singles.tile([P, N], bf16)
        vTs = singles.tile([D, N], bf16)
        FT = 512
        for t in range(N // FT):
            sl = slice(t * FT, (t + 1) * FT)
            for g, dst in enumerate((qT4, kT4)):
                ps = st_pool.tile([P, FT], f32, tag="st", name=f"qk{g}_{t}")
                for r in range(4):
                    for c in range(2):
                        nc.tensor.matmul(
                            ps[32 * r:32 * (r + 1), :],
                            lhsT=ws[:, c, g * D:(g + 1) * D],
                            rhs=xs[:, c, sl],
                            start=(c == 0), stop=(c == 1),
                            tile_position=(0, 32 * r))
                if t % 2 == 0:
                    nc.vector.tensor_scalar_add(dst[:, sl], ps[:],
                                                bq4[:, g:g + 1])
                else:
                    nc.scalar.activation(dst[:, sl], ps[:], AF.Identity,
                                         bias=bq4[:, g:g + 1])
            psv = acc_pool.tile([D, FT], f32, tag="acc", name=f"v_{t}")
            for c in range(2):
                nc.tensor.matmul(psv[:], lhsT=ws[:, c, 2 * D:3 * D],
                                 rhs=xs[:, c, sl],
                                 start=(c == 0), stop=(c == 1))
            if t % 2 == 0:
                nc.scalar.activation(vTs[:, sl], psv[:], AF.Identity,
                                     bias=bq4[0:D, 2:3])
            else:
                nc.vector.tensor_scalar_add(vTs[:, sl], psv[:],
                                            bq4[0:D, 2:3])

        # V -> [128 j, 32 d] tiles via DMA xbar transposes
        vsb = singles.tile([P, NT, D], bf16)
        for j in range(NT):
            nc.sync.dma_start_transpose(vsb[:, j, :],
                                        vTs[:, j * P:(j + 1) * P])

        # ---------------- attention pass loop ----------------
        # a2a staging: shard d = my head's O^T for dest core d's rows.
        agi = dram.tile([NCORES, D, W], bf16, name="agi", tag="agi")
        ago = dram.tile([NCORES, D, W], bf16, name="ago", tag="ago")

        ptqs, ptqis = [None, None], [None, None]
        acc = [None]
        # projection PSUM, pre-allocated so HAM-warming filler matmuls can
        # target it harmlessly during the pass loop
        yt = prj_pool.tile([P, 2, W], f32, tag="prj")

        def filler(n):
            # tiny matmuls that keep the PE activity monitor at full clock
            for _ in range(n):
                nc.tensor.matmul(yt[0:1, 0, :], lhsT=ones1[0:1, 0:1],
                                 rhs=qT4[0:1, 0:W], start=True, stop=True,
                                 skip_group_check=True)

        def emit_epilogue(pp):
            # pass pp: acc holds O_even^T (parts 0-31), O_odd^T (32-63),
            # r_even (64), r_odd (96), all [_, 512] f32.
            a = acc[0]
            # staging rows mirror the PSUM layout (32-aligned bases only)
            eo = sm_pool.tile([P, W], bf16, tag="eo", name=f"eo{pp}")
            nc.vector.memset(eo[:], 0.0)
            nc.vector.tensor_copy(eo[0:65, :], a[0:65, :])
            nc.scalar.activation(eo[96:97, :], a[96:97, :], AF.Copy)
            ot = sm_pool.tile([P, 4, P], bf16, tag="ot", name=f"ot{pp}")
            for c in range(4):
                eng = nc.sync if c % 2 == 0 else nc.scalar
                eng.dma_start_transpose(ot[:, c, :],
                                        eo[:, c * P:(c + 1) * P])
            # r = r_even + r_odd; rinv = 1/r  (per-partition, cheap)
            rs = sm_pool.tile([P, 4, 1], f32, tag="rs", name=f"rs{pp}")
            nc.vector.scalar_tensor_tensor(
                rs[:], ot[:, :, 64:65], 1.0, ot[:, :, 96:97],
                op0=ALU.mult, op1=ALU.add)
            ri = sm_pool.tile([P, 4, 1], f32, tag="ri", name=f"ri{pp}")
            nc.vector.reciprocal(ri[:], rs[:])
            # O = (O_even + O_odd) * rinv -> bf16
            osum = sm_pool.tile([P, 4, D], f32, tag="os", name=f"os{pp}")
            nc.vector.scalar_tensor_tensor(
                osum[:], ot[:, :, 0:D], 1.0, ot[:, :, D:2 * D],
                op0=ALU.mult, op1=ALU.add)
            obq = sm_pool.tile([P, 4, D], bf16, tag="obq", name=f"obq{pp}")
            for c in range(4):
                nc.scalar.activation(obq[:, c, :], osum[:, c, :], AF.Copy,
                                     scale=ri[:, c, 0:1])
            # transpose to O^T [ (c d), i ] and stage the a2a shard
            otb = sm_pool.tile([P, P], bf16, tag="otb", name=f"otb{pp}")
            nc.scalar.dma_start_transpose(
                otb[:], obq.rearrange("p a b -> p (a b)"))
            for c in range(4):
                nc.sync.dma_start(out=agi[pp][:, c * P:(c + 1) * P],
                                  in_=otb[c * D:(c + 1) * D, :])

        for p in range(NPASS + 1):
            if p < NPASS:
                ptqs[p % 2] = pt_pool.tile([P, N_SC_J, W], bf16, tag="pt",
                                           name=f"ptq{p}")
                ptqis[p % 2] = pt_pool.tile([P, N_DV_J, W], bf16, tag="pti",
                                            name=f"ptqi{p}")
            isl = slice((p % NPASS) * W, (p % NPASS) * W + W)
            for s in range(NSLOT):
                if p < NPASS:
                    ptq, ptqi = ptqs[p % 2], ptqis[p % 2]
                    st = st_pool.tile([P, 2, W], f32, tag="st",
                                      name=f"st{p}_{s}")
                    for r in range(2):
                        j = 2 * s + r
                        nc.tensor.matmul(
                            st[:, r, :],
                            lhsT=kT4[32 * r:32 * (r + 1),
                                     j * P:(j + 1) * P],
                            rhs=qT4[32 * r:32 * (r + 1), isl],
                            start=True, stop=True,
                            tile_position=(32 * r, 0))
                    if EXP_DVE[s]:
                        ro = _dv_rows[s]
                        nc.vector.tensor_scalar(
                            ptqi[:, ro:ro + 2, :].bitcast(i16), st[:],
                            scalar1=float(FE_A), scalar2=float(FE_B),
                            op0=ALU.mult, op1=ALU.add)
                    else:
                        ro = _sc_rows[s]
                        nc.scalar.activation(ptq[:, ro:ro + 2, :], st[:],
                                             AF.Exp, scale=float(SCALE))
                if p > 0:
                    pptq, pptqi = ptqs[(p - 1) % 2], ptqis[(p - 1) % 2]
                    if s == 0:
                        acc[0] = acc_pool.tile([P, W], f32, tag="acc",
                                               name=f"acc{p - 1}")
                    a = acc[0]
                    rhs2 = []
                    for r in range(2):
                        j = 2 * s + r
                        if EXP_DVE[s]:
                            rhs2.append(pptqi[:, _dv_rows[s] + r, :])
                        else:
                            rhs2.append(pptq[:, _sc_rows[s] + r, :])
                    st_flag = dict(start=(s == 0), stop=(s == NSLOT - 1))
                    nc.tensor.matmul(a[0:D, :], lhsT=vsb[:, 2 * s, :],
                                     rhs=rhs2[0], tile_position=(0, 0),
                                     **st_flag)
                    nc.tensor.matmul(a[D:2 * D, :], lhsT=vsb[:, 2 * s + 1, :],
                                     rhs=rhs2[1], tile_position=(0, 32),
                                     **st_flag)
                    nc.tensor.matmul(a[64:65, :], lhsT=ones1[:],
                                     rhs=rhs2[0], tile_position=(0, 64),
                                     **st_flag)
                    nc.tensor.matmul(a[96:97, :], lhsT=ones1[:],
                                     rhs=rhs2[1], tile_position=(0, 96),
                                     **st_flag)
                filler(FILL_N)
            if p > 0:
                emit_epilogue(p - 1)
                filler(4)

        # ---------------- exchange + output projection ----------------
        nc.gpsimd.collective_compute(
            "AllToAll", ALU.bypass,
            replica_groups=[list(range(NCORES))],
            ins=[agi[:].opt()], outs=[ago[:].opt()])
        ago_flat = ago.rearrange("c d r -> (c d) r")
        osl = singles.tile([P, 2, W], bf16)
        for c in range(2):
            nc.sync.dma_start(out=osl[:, c, :],
                              in_=ago_flat[c * P:(c + 1) * P, :])
        for dc in range(2):
            for cin in range(2):
                nc.tensor.matmul(yt[:, dc, :],
                                 lhsT=wos[:, cin, dc * P:(dc + 1) * P],
                                 rhs=osl[:, cin, :],
                                 start=(cin == 0), stop=False)
            nc.tensor.matmul(yt[:, dc, :],
                             lhsT=bos[0:1, dc * P:(dc + 1) * P],
                             rhs=onesw[:], start=False, stop=True)
        yts = singles.tile([P, 2, W], f32)
        nc.vector.tensor_copy(yts[:, 0, :], yt[:, 0, :])
        nc.scalar.activation(yts[:, 1, :], yt[:, 1, :], AF.Copy)
        for dc in range(2):
            nc.sync.dma_start(out=outT[dc * P:(dc + 1) * P, :],
                              in_=yts[:, dc, :])

    nc.compile()
    return nc


def _get_built():
    global _BUILT
    if _BUILT is None:
        _BUILT = _build()
    return _BUILT


def make_in_maps(x, w_qkv, b_qkv, w_out, b_out):
    import ml_dtypes
    bf = ml_dtypes.bfloat16

    x = np.asarray(x, dtype=np.float32)
    w_qkv = np.asarray(w_qkv, dtype=np.float32)
    b_qkv = np.asarray(b_qkv, dtype=np.float32)
    w_out = np.asarray(w_out, dtype=np.float32)
    b_out = np.asarray(b_out, dtype=np.float32)

    xT = np.ascontiguousarray(x.T).astype(bf)
    wq3 = w_qkv.reshape(DIM, 3, H, D)       # [in, (q|k|v), head, d]
    bq3 = b_qkv.reshape(3, H, D)
    wob = np.ascontiguousarray(w_out).astype(bf)
    bob = np.ascontiguousarray(b_out.reshape(1, DIM)).astype(bf)
    in_maps = []
    for h in range(NCORES):
        # bq4[32r+d, g] = b_qkv[gate g, head h, dim d], band-replicated
        bq4 = np.ascontiguousarray(
            np.tile(bq3[:, h, :], (1, 4)).reshape(3, P).T).astype(np.float32)
        in_maps.append({
            "xb": xT,
            "wqkv": np.ascontiguousarray(
                wq3[:, :, h, :].reshape(DIM, 3 * D)).astype(bf),
            "bqkv4": bq4,
            "wo": wob,
            "bo": bob,
        })
    return in_maps


def assemble_output(results):
    # core h returns y^T [256, 512] for rows [512h, 512h+512)
    return np.concatenate(
        [np.ascontiguousarray(results[i]["outT"]).T.astype(np.float32)
         for i in range(NCORES)], axis=0)


def kernel(x, w_qkv, b_qkv, w_out, b_out):
    from concourse.bass_utils import run_bass_kernel_spmd

    nc = _get_built()
    in_maps = make_in_maps(x, w_qkv, b_qkv, w_out, b_out)
    res = run_bass_kernel_spmd(nc, in_maps, core_ids=list(range(NCORES)))
    return assemble_output(res.results)


# revision 17
# speedup vs baseline: 1.5062x; 1.5062x over previous
"""Distributed Trainium2 (8 NeuronCore) multi-head attention kernel, v2.

Problem: y = softmax((x Wq)(x Wk)^T * DIM**-0.5) (x Wv) Wo + bo
  x: [4096, 256], 8 heads of dim 32, scale by full-dim**-0.5 (1/16).

Sharding: head-parallel, one head per core. v2 restructures the matmul
orientations so the PE never loads large stationary operands:

  - QKV projection with W stationary; q^T/k^T are produced REPLICATED in
    four 32-partition bands via PE column tiling (4 concurrent matmuls
    share the x stream), so scores can use PE row tiling.
  - Scores S^T[j, i-block]: kT stationary [32,128] per j-tile, qT moving
    512 wide; two j-tiles run concurrently in PE row bands {0,32}.
  - exp: ScalarE spline (10/16 slots) + DVE Schraudolph bf16 bit-trick
    (6/16 slots), both reading f32 PSUM score slots.
  - AV: V stationary (tiny, 32 cols) streaming P^T 512 wide; four
    concurrent col-tiled matmuls per j-pair produce O_even^T/O_odd^T in
    PSUM partition bands 0-31/32-63 plus softmax row-sums in bands 64/96
    via ones-column stationaries. The halves are summed after a DMA-xbar
    transpose into [i, d] orientation where the softmax division is a
    cheap per-partition reciprocal + scale.
  - Output exchange: one AllToAll of O^T (256KB bf16 per core), then the
    projection with Wout stationary and O^T moving produces y^T; the host
    transposes each core's [256, 512] slice back to rows.
"""

import numpy as np

P = 128          # partitions
N = 4096         # sequence length
DIM = 256        # model dim
H = 8            # heads == cores
D = DIM // H     # head dim, 32
NCORES = 8
W = 512          # i-columns per pass
NPASS = N // W   # 8 passes; pass p covers dest core p's rows
NT = N // P      # 32 j-tiles
NSLOT = NT // 2  # 16 slots of 2 j-tiles per pass
RPC = N // NCORES
SCALE = DIM ** -0.5

# Schraudolph bf16 fast-exp: bits(exp(s*SCALE)) ~= s*FE_A + FE_B (int16)
FE_A = 128.0 * SCALE * 1.4426950408889634
FE_B = 16256.0 - 4.6

# Per-slot exp engine: True = DVE Schraudolph, False = ScalarE spline.
EXP_DVE = (False, False, True, False, True, False, True, False, True,
           False, True, False, True, False, True, False)
assert len(EXP_DVE) == NSLOT
# row index of each slot's 2 j-rows inside its engine's P^T tile
_sc_rows, _dv_rows = {}, {}
for _s, _d in enumerate(EXP_DVE):
    if _d:
        _dv_rows[_s] = 2 * len(_dv_rows)
    else:
        _sc_rows[_s] = 2 * len(_sc_rows)
N_SC_J = 2 * len(_sc_rows)   # 20
N_DV_J = 2 * len(_dv_rows)   # 12

_BUILT = None


def _build():
    from contextlib import ExitStack

    import concourse.mybir as mybir
    import concourse.tile as tile
    from concourse import bacc

    f32 = mybir.dt.float32
    bf16 = mybir.dt.bfloat16
    i16 = mybir.dt.int16
    AF = mybir.ActivationFunctionType
    ALU = mybir.AluOpType

    nc = bacc.Bacc("TRN2", target_bir_lowering=False, debug=False,
                   num_devices=NCORES)
    xb = nc.dram_tensor("xb", [DIM, N], bf16, kind="ExternalInput")
    wqkv = nc.dram_tensor("wqkv", [DIM, 3 * D], bf16, kind="ExternalInput")
    bqkv4 = nc.dram_tensor("bqkv4", [P, 3], f32, kind="ExternalInput")
    wo = nc.dram_tensor("wo", [DIM, DIM], bf16, kind="ExternalInput")
    bo = nc.dram_tensor("bo", [1, DIM], bf16, kind="ExternalInput")
    outT = nc.dram_tensor("outT", [DIM, RPC], f32, kind="ExternalOutput")

    with tile.TileContext(nc) as tc, ExitStack() as ctx:
        singles = ctx.enter_context(tc.tile_pool(name="singles", bufs=1))
        sm_pool = ctx.enter_context(tc.tile_pool(name="sm", bufs=3))
        pt_pool = ctx.enter_context(tc.tile_pool(name="ptp", bufs=2))
        st_pool = ctx.enter_context(
            tc.tile_pool(name="stp", bufs=2, space="PSUM"))
        acc_pool = ctx.enter_context(
            tc.tile_pool(name="accp", bufs=2, space="PSUM"))
        prj_pool = ctx.enter_context(
            tc.tile_pool(name="prjp", bufs=1, space="PSUM"))
        dram = ctx.enter_context(
            tc.tile_pool(name="dram", bufs=1, space="DRAM"))

        # ---------------- input loads ----------------
        xs = singles.tile([P, 2, N], bf16)
        for c in range(2):
            for t in range(4):
                sl = slice(t * (N // 4), (t + 1) * (N // 4))
                nc.sync.dma_start(out=xs[:, c, sl], in_=xb[c * P:(c + 1) * P, sl])

        ws = singles.tile([P, 2, 3 * D], bf16)
        for c in range(2):
            nc.sync.dma_start(out=ws[:, c, :], in_=wqkv[c * P:(c + 1) * P, :])
        bq4 = singles.tile([P, 3], f32)
        nc.sync.dma_start(out=bq4[:], in_=bqkv4[:, :])
        wos = singles.tile([P, 2, DIM], bf16)
        for c in range(2):
            nc.sync.dma_start(out=wos[:, c, :], in_=wo[c * P:(c + 1) * P, :])
        bos = singles.tile([1, DIM], bf16)
        nc.sync.dma_start(out=bos[:], in_=bo[:, :])

        ones1 = singles.tile([P, 1], bf16)
        nc.vector.memset(ones1[:], 1.0)
        onesw = singles.tile([1, W], bf16)
        nc.vector.memset(onesw[:], 1.0)


        # ------- QKV projection -------
        # qT4/kT4: [128, 4096] bf16, four replicated 32-row bands
        # (col-tiled matmuls share the x stream). vTs: [32, 4096] bf16.
        qT4 = singles.tile([P, N], bf16)
        kT4 = singles.tile([P, N], bf16)
        vTs = singles.tile([D, N], bf16)
        FT = 512
        for t in range(N // FT):
            sl = slice(t * FT, (t + 1) * FT)
            for g, dst in enumerate((qT4, kT4)):
                ps = st_pool.tile([P, FT], f32, tag="st", name=f"qk{g}_{t}")
                for r in range(4):
                    for c in range(2):
                        nc.tensor.matmul(
                            ps[32 * r:32 * (r + 1), :],
                            lhsT=ws[:, c, g * D:(g + 1) * D],
                            rhs=xs[:, c, sl],
                            start=(c == 0), stop=(c == 1),
                            tile_position=(0, 32 * r))
                if t % 2 == 0:
                    nc.vector.tensor_scalar_add(dst[:, sl], ps[:],
                                                bq4[:, g:g + 1])
                else:
                    nc.scalar.activation(dst[:, sl], ps[:], AF.Identity,
                                         bias=bq4[:, g:g + 1])
            psv = acc_pool.tile([D, FT], f32, tag="acc", name=f"v_{t}")
            for c in range(2):
                nc.tensor.matmul(psv[:], lhsT=ws[:, c, 2 * D:3 * D],
                                 rhs=xs[:, c, sl],
                                 start=(c == 0), stop=(c == 1))
            if t % 2 == 0:
                nc.scalar.activation(vTs[:, sl], psv[:], AF.Identity,
                                     bias=bq4[0:D, 2:3])
            else:
                nc.vector.tensor_scalar_add(vTs[:, sl], psv[:],
                                            bq4[0:D, 2:3])

        # V -> [128 j, 32 d] tiles via DMA xbar transposes
        vsb = singles.tile([P, NT, D], bf16)
        for j in range(NT):
            nc.sync.dma_start_transpose(vsb[:, j, :],
                                        vTs[:, j * P:(j + 1) * P])

        # ---------------- attention pass loop ----------------
        # a2a staging: shard d = my head's O^T for dest core d's rows.
        agi = dram.tile([NCORES, D, W], bf16, name="agi", tag="agi")
        ago = dram.tile([NCORES, D, W], bf16, name="ago", tag="ago")

        ptqs, ptqis = [None, None], [None, None]
        acc = [None]

        def emit_epilogue(pp):
            # pass pp: acc holds O_even^T (parts 0-31), O_odd^T (32-63),
            # r_even (64), r_odd (96), all [_, 512] f32.
            a = acc[0]
            # staging rows mirror the PSUM layout (32-aligned bases only)
            eo = sm_pool.tile([P, W], bf16, tag="eo", name=f"eo{pp}")
            nc.gpsimd.memset(eo[:], 0.0)
            nc.vector.tensor_copy(eo[0:65, :], a[0:65, :])
            nc.scalar.activation(eo[96:97, :], a[96:97, :], AF.Copy)
            ot = sm_pool.tile([P, 4, P], bf16, tag="ot", name=f"ot{pp}")
            for c in range(4):
                nc.sync.dma_start_transpose(ot[:, c, :],
                                            eo[:, c * P:(c + 1) * P])
            # r = r_even + r_odd; rinv = 1/r  (per-partition, cheap)
            rs = sm_pool.tile([P, 4, 1], f32, tag="rs", name=f"rs{pp}")
            nc.vector.scalar_tensor_tensor(
                rs[:], ot[:, :, 64:65], 1.0, ot[:, :, 96:97],
                op0=ALU.mult, op1=ALU.add)
            ri = sm_pool.tile([P, 4, 1], f32, tag="ri", name=f"ri{pp}")
            nc.vector.reciprocal(ri[:], rs[:])
            # O = (O_even + O_odd) * rinv -> bf16
            osum = sm_pool.tile([P, 4, D], f32, tag="os", name=f"os{pp}")
            nc.vector.scalar_tensor_tensor(
                osum[:], ot[:, :, 0:D], 1.0, ot[:, :, D:2 * D],
                op0=ALU.mult, op1=ALU.add)
            obq = sm_pool.tile([P, 4, D], bf16, tag="obq", name=f"obq{pp}")
            for c in range(4):
                nc.scalar.activation(obq[:, c, :], osum[:, c, :], AF.Copy,
                                     scale=ri[:, c, 0:1])
            # transpose to O^T [ (c d), i ] and stage the a2a shard
            otb = sm_pool.tile([P, P], bf16, tag="otb", name=f"otb{pp}")
            nc.scalar.dma_start_transpose(
                otb[:], obq.rearrange("p a b -> p (a b)"))
            for c in range(4):
                nc.sync.dma_start(out=agi[pp][:, c * P:(c + 1) * P],
                                  in_=otb[c * D:(c + 1) * D, :])

        for p in range(NPASS + 1):
            if p < NPASS:
                ptqs[p % 2] = pt_pool.tile([P, N_SC_J, W], bf16, tag="pt",
                                           name=f"ptq{p}")
                ptqis[p % 2] = pt_pool.tile([P, N_DV_J, W], bf16, tag="pti",
                                            name=f"ptqi{p}")
            isl = slice((p % NPASS) * W, (p % NPASS) * W + W)
            for s in range(NSLOT):
                if p < NPASS:
                    ptq, ptqi = ptqs[p % 2], ptqis[p % 2]
                    st = st_pool.tile([P, 2, W], f32, tag="st",
                                      name=f"st{p}_{s}")
                    for r in range(2):
                        j = 2 * s + r
                        b = 2 * (s % 2) + r   # row band: 4-way concurrency
                        nc.tensor.matmul(
                            st[:, r, :],
                            lhsT=kT4[32 * b:32 * (b + 1),
                                     j * P:(j + 1) * P],
                            rhs=qT4[32 * b:32 * (b + 1), isl],
                            start=True, stop=True,
                            tile_position=(32 * b, 0))
                    if EXP_DVE[s]:
                        ro = _dv_rows[s]
                        nc.vector.tensor_scalar(
                            ptqi[:, ro:ro + 2, :].bitcast(i16), st[:],
                            scalar1=float(FE_A), scalar2=float(FE_B),
                            op0=ALU.mult, op1=ALU.add)
                    else:
                        ro = _sc_rows[s]
                        nc.scalar.activation(ptq[:, ro:ro + 2, :], st[:],
                                             AF.Exp, scale=float(SCALE))
                if p > 0:
                    pptq, pptqi = ptqs[(p - 1) % 2], ptqis[(p - 1) % 2]
                    if s == 0:
                        acc[0] = acc_pool.tile([P, W], f32, tag="acc",
                                               name=f"acc{p - 1}")
                    a = acc[0]
                    rhs2 = []
                    for r in range(2):
                        j = 2 * s + r
                        if EXP_DVE[s]:
                            rhs2.append(pptqi[:, _dv_rows[s] + r, :])
                        else:
                            rhs2.append(pptq[:, _sc_rows[s] + r, :])
                    st_flag = dict(start=(s == 0), stop=(s == NSLOT - 1))
                    nc.tensor.matmul(a[0:D, :], lhsT=vsb[:, 2 * s, :],
                                     rhs=rhs2[0], tile_position=(0, 0),
                                     **st_flag)
                    nc.tensor.matmul(a[D:2 * D, :], lhsT=vsb[:, 2 * s + 1, :],
                                     rhs=rhs2[1], tile_position=(0, 32),
                                     **st_flag)
                    nc.tensor.matmul(a[64:65, :], lhsT=ones1[:],
                                     rhs=rhs2[0], tile_position=(0, 64),
                                     **st_flag)
                    nc.tensor.matmul(a[96:97, :], lhsT=ones1[:],
                                     rhs=rhs2[1], tile_position=(0, 96),
                                     **st_flag)
            if p > 0:
                emit_epilogue(p - 1)

        # ---------------- exchange + output projection ----------------
        nc.gpsimd.collective_compute(
            "AllToAll", ALU.bypass,
            replica_groups=[list(range(NCORES))],
            ins=[agi[:].opt()], outs=[ago[:].opt()])
        ago_flat = ago.rearrange("c d r -> (c d) r")
        osl = singles.tile([P, 2, W], bf16)
        for c in range(2):
            nc.sync.dma_start(out=osl[:, c, :],
                              in_=ago_flat[c * P:(c + 1) * P, :])
        yt = prj_pool.tile([P, 2, W], f32, tag="prj")
        for dc in range(2):
            for cin in range(2):
                nc.tensor.matmul(yt[:, dc, :],
                                 lhsT=wos[:, cin, dc * P:(dc + 1) * P],
                                 rhs=osl[:, cin, :],
                                 start=(cin == 0), stop=False)
            nc.tensor.matmul(yt[:, dc, :],
                             lhsT=bos[0:1, dc * P:(dc + 1) * P],
                             rhs=onesw[:], start=False, stop=True)
        yts = singles.tile([P, 2, W], f32)
        nc.vector.tensor_copy(yts[:, 0, :], yt[:, 0, :])
        nc.scalar.activation(yts[:, 1, :], yt[:, 1, :], AF.Copy)
        for dc in range(2):
            nc.sync.dma_start(out=outT[dc * P:(dc + 1) * P, :],
                              in_=yts[:, dc, :])

    nc.compile()
    return nc


def _get_built():
    global _BUILT
    if _BUILT is None:
        _BUILT = _build()
    return _BUILT


def make_in_maps(x, w_qkv, b_qkv, w_out, b_out):
    import ml_dtypes
    bf = ml_dtypes.bfloat16

    x = np.asarray(x, dtype=np.float32)
    w_qkv = np.asarray(w_qkv, dtype=np.float32)
    b_qkv = np.asarray(b_qkv, dtype=np.float32)
    w_out = np.asarray(w_out, dtype=np.float32)
    b_out = np.asarray(b_out, dtype=np.float32)

    xT = np.ascontiguousarray(x.T).astype(bf)
    wq3 = w_qkv.reshape(DIM, 3, H, D)       # [in, (q|k|v), head, d]
    bq3 = b_qkv.reshape(3, H, D)
    wob = np.ascontiguousarray(w_out).astype(bf)
    bob = np.ascontiguousarray(b_out.reshape(1, DIM)).astype(bf)
    in_maps = []
    for h in range(NCORES):
        # bq4[32r+d, g] = b_qkv[gate g, head h, dim d], band-replicated
        bq4 = np.ascontiguousarray(
            np.tile(bq3[:, h, :], (1, 4)).reshape(3, P).T).astype(np.float32)
        in_maps.append({
            "xb": xT,
            "wqkv": np.ascontiguousarray(
                wq3[:, :, h, :].reshape(DIM, 3 * D)).astype(bf),
            "bqkv4": bq4,
            "wo": wob,
            "bo": bob,
        })
    return in_maps


def assemble_output(results):
    # core h returns y^T [256, 512] for rows [512h, 512h+512)
    return np.concatenate(
        [np.ascontiguousarray(results[i]["outT"]).T.astype(np.float32)
         for i in range(NCORES)], axis=0)


def kernel(x, w_qkv, b_qkv, w_out, b_out):
    from concourse.bass_utils import run_bass_kernel_spmd

    nc = _get_built()
    in_maps = make_in_maps(x, w_qkv, b_qkv, w_out, b_out)
    res = run_bass_kernel_spmd(nc, in_maps, core_ids=list(range(NCORES)))
    return assemble_output(res.results)


# revision 19
# speedup vs baseline: 1.6513x; 1.0963x over previous
"""Distributed Trainium2 (8 NeuronCore) multi-head attention kernel, v2.

Problem: y = softmax((x Wq)(x Wk)^T * DIM**-0.5) (x Wv) Wo + bo
  x: [4096, 256], 8 heads of dim 32, scale by full-dim**-0.5 (1/16).

Sharding: head-parallel, one head per core. v2 restructures the matmul
orientations so the PE never loads large stationary operands:

  - QKV projection with W stationary; q^T/k^T are produced REPLICATED in
    four 32-partition bands via PE column tiling (4 concurrent matmuls
    share the x stream), so scores can use PE row tiling.
  - Scores S^T[j, i-block]: kT stationary [32,128] per j-tile, qT moving
    512 wide; two j-tiles run concurrently in PE row bands {0,32}.
  - exp: ScalarE spline (10/16 slots) + DVE Schraudolph bf16 bit-trick
    (6/16 slots), both reading f32 PSUM score slots.
  - AV: V stationary (tiny, 32 cols) streaming P^T 512 wide; four
    concurrent col-tiled matmuls per j-pair produce O_even^T/O_odd^T in
    PSUM partition bands 0-31/32-63 plus softmax row-sums in bands 64/96
    via ones-column stationaries. The halves are summed after a DMA-xbar
    transpose into [i, d] orientation where the softmax division is a
    cheap per-partition reciprocal + scale.
  - Output exchange: one AllToAll of O^T (256KB bf16 per core), then the
    projection with Wout stationary and O^T moving produces y^T; the host
    transposes each core's [256, 512] slice back to rows.
"""

import numpy as np

P = 128          # partitions
N = 4096         # sequence length
DIM = 256        # model dim
H = 8            # heads == cores
D = DIM // H     # head dim, 32
NCORES = 8
W = 512          # i-columns per pass
NPASS = N // W   # 8 passes; pass p covers dest core p's rows
NT = N // P      # 32 j-tiles
NSLOT = NT // 2  # 16 slots of 2 j-tiles per pass
RPC = N // NCORES
SCALE = DIM ** -0.5

# Schraudolph bf16 fast-exp: bits(exp(s*SCALE)) ~= s*FE_A + FE_B (int16)
FE_A = 128.0 * SCALE * 1.4426950408889634
FE_B = 16256.0 - 4.6

# Per-slot exp engine: True = DVE Schraudolph, False = ScalarE spline.
EXP_DVE = (False, True, False, True, False, True, False, True,
           False, True, False, True, False, True, False, False)
assert len(EXP_DVE) == NSLOT
# row index of each slot's 2 j-rows inside its engine's P^T tile
_sc_rows, _dv_rows = {}, {}
for _s, _d in enumerate(EXP_DVE):
    if _d:
        _dv_rows[_s] = 2 * len(_dv_rows)
    else:
        _sc_rows[_s] = 2 * len(_sc_rows)
N_SC_J = 2 * len(_sc_rows)   # 20
N_DV_J = 2 * len(_dv_rows)   # 12

_BUILT = None


def _build():
    from contextlib import ExitStack

    import concourse.mybir as mybir
    import concourse.tile as tile
    from concourse import bacc

    f32 = mybir.dt.float32
    bf16 = mybir.dt.bfloat16
    i16 = mybir.dt.int16
    AF = mybir.ActivationFunctionType
    ALU = mybir.AluOpType

    nc = bacc.Bacc("TRN2", target_bir_lowering=False, debug=False,
                   num_devices=NCORES)
    xb = nc.dram_tensor("xb", [DIM, N], bf16, kind="ExternalInput")
    wqkv = nc.dram_tensor("wqkv", [DIM, 3 * D], bf16, kind="ExternalInput")
    bqkv4 = nc.dram_tensor("bqkv4", [P, 3], f32, kind="ExternalInput")
    wo = nc.dram_tensor("wo", [DIM, DIM], bf16, kind="ExternalInput")
    bo = nc.dram_tensor("bo", [1, DIM], bf16, kind="ExternalInput")
    outT = nc.dram_tensor("outT", [DIM, RPC], f32, kind="ExternalOutput")

    with tile.TileContext(nc) as tc, ExitStack() as ctx:
        singles = ctx.enter_context(tc.tile_pool(name="singles", bufs=1))
        sm_pool = ctx.enter_context(tc.tile_pool(name="sm", bufs=3))
        pt_pool = ctx.enter_context(tc.tile_pool(name="ptp", bufs=2))
        st_pool = ctx.enter_context(
            tc.tile_pool(name="stp", bufs=2, space="PSUM"))
        acc_pool = ctx.enter_context(
            tc.tile_pool(name="accp", bufs=2, space="PSUM"))
        prj_pool = ctx.enter_context(
            tc.tile_pool(name="prjp", bufs=1, space="PSUM"))
        dram = ctx.enter_context(
            tc.tile_pool(name="dram", bufs=1, space="DRAM"))

        # ---------------- input loads ----------------
        xs = singles.tile([P, 2, N], bf16)
        for c in range(2):
            for t in range(4):
                sl = slice(t * (N // 4), (t + 1) * (N // 4))
                nc.sync.dma_start(out=xs[:, c, sl], in_=xb[c * P:(c + 1) * P, sl])

        ws = singles.tile([P, 2, 3 * D], bf16)
        for c in range(2):
            nc.sync.dma_start(out=ws[:, c, :], in_=wqkv[c * P:(c + 1) * P, :])
        bq4 = singles.tile([P, 3], f32)
        nc.sync.dma_start(out=bq4[:], in_=bqkv4[:, :])
        wos = singles.tile([P, 2, DIM], bf16)
        for c in range(2):
            nc.sync.dma_start(out=wos[:, c, :], in_=wo[c * P:(c + 1) * P, :])
        bos = singles.tile([1, DIM], bf16)
        nc.sync.dma_start(out=bos[:], in_=bo[:, :])

        ones1 = singles.tile([P, 1], bf16)
        nc.vector.memset(ones1[:], 1.0)
        onesw = singles.tile([1, W], bf16)
        nc.vector.memset(onesw[:], 1.0)


        # ------- QKV projection -------
        # qT4/kT4: [128, 4096] bf16, four replicated 32-row bands
        # (col-tiled matmuls share the x stream). vTs: [32, 4096] bf16.
        qT4 = singles.tile([P, N], bf16)
        kT4 = singles.tile([P, N], bf16)
        vTs = singles.tile([D, N], bf16)
        FT = 512
        for t in range(N // FT):
            sl = slice(t * FT, (t + 1) * FT)
            for g, dst in enumerate((qT4, kT4)):
                ps = st_pool.tile([P, FT], f32, tag="st", name=f"qk{g}_{t}")
                for r in range(4):
                    for c in range(2):
                        nc.tensor.matmul(
                            ps[32 * r:32 * (r + 1), :],
                            lhsT=ws[:, c, g * D:(g + 1) * D],
                            rhs=xs[:, c, sl],
                            start=(c == 0), stop=(c == 1),
                            tile_position=(0, 32 * r))
                if t % 2 == 0:
                    nc.vector.tensor_scalar_add(dst[:, sl], ps[:],
                                                bq4[:, g:g + 1])
                else:
                    nc.scalar.activation(dst[:, sl], ps[:], AF.Identity,
                                         bias=bq4[:, g:g + 1])
            psv = acc_pool.tile([D, FT], f32, tag="acc", name=f"v_{t}")
            for c in range(2):
                nc.tensor.matmul(psv[:], lhsT=ws[:, c, 2 * D:3 * D],
                                 rhs=xs[:, c, sl],
                                 start=(c == 0), stop=(c == 1))
            if t % 2 == 0:
                nc.scalar.activation(vTs[:, sl], psv[:], AF.Identity,
                                     bias=bq4[0:D, 2:3])
            else:
                nc.vector.tensor_scalar_add(vTs[:, sl], psv[:],
                                            bq4[0:D, 2:3])

        # V -> [128 j, 32 d] tiles via PE transposes
        from concourse.masks import make_identity
        ident = singles.tile([D, D], bf16)
        make_identity(nc, ident[:])
        vsb = singles.tile([P, NT, D], bf16)
        vt = st_pool.tile([P, NT * D], bf16, tag="st", name="vt")
        for j in range(NT):
            nc.tensor.transpose(vt[:, j * D:(j + 1) * D],
                                vTs[:, j * P:(j + 1) * P], ident[:])
        nc.scalar.activation(vsb.rearrange("p a b -> p (a b)"), vt[:],
                             AF.Copy)

        # ---------------- attention pass loop ----------------
        # a2a staging: shard d = my head's O^T for dest core d's rows.
        agi = dram.tile([NCORES, D, W], bf16, name="agi", tag="agi")
        ago = dram.tile([NCORES, D, W], bf16, name="ago", tag="ago")

        ptqs, ptqis = [None, None], [None, None]
        acc = [None]

        def emit_epilogue(pp):
            # pass pp: acc holds O_even^T (parts 0-31), O_odd^T (32-63),
            # r_even (64), r_odd (96), all [_, 512] f32.
            a = acc[0]
            # staging rows mirror the PSUM layout (32-aligned bases only)
            eo = sm_pool.tile([P, W], bf16, tag="eo", name=f"eo{pp}")
            nc.gpsimd.memset(eo[:], 0.0)
            nc.vector.tensor_copy(eo[0:65, :], a[0:65, :])
            nc.scalar.activation(eo[96:97, :], a[96:97, :], AF.Copy)
            ot = sm_pool.tile([P, 4, P], bf16, tag="ot", name=f"ot{pp}")
            for c in range(4):
                eng = nc.sync if c % 2 == 0 else nc.scalar
                eng.dma_start_transpose(ot[:, c, :],
                                        eo[:, c * P:(c + 1) * P])
            # r = r_even + r_odd; rinv = 1/r  (per-partition, cheap)
            rs = sm_pool.tile([P, 4, 1], f32, tag="rs", name=f"rs{pp}")
            nc.vector.scalar_tensor_tensor(
                rs[:], ot[:, :, 64:65], 1.0, ot[:, :, 96:97],
                op0=ALU.mult, op1=ALU.add)
            ri = sm_pool.tile([P, 4, 1], f32, tag="ri", name=f"ri{pp}")
            nc.vector.reciprocal(ri[:], rs[:])
            # O = (O_even + O_odd) * rinv -> bf16
            osum = sm_pool.tile([P, 4, D], f32, tag="os", name=f"os{pp}")
            nc.vector.scalar_tensor_tensor(
                osum[:], ot[:, :, 0:D], 1.0, ot[:, :, D:2 * D],
                op0=ALU.mult, op1=ALU.add)
            obq = sm_pool.tile([P, 4, D], bf16, tag="obq", name=f"obq{pp}")
            for c in range(4):
                nc.scalar.activation(obq[:, c, :], osum[:, c, :], AF.Copy,
                                     scale=ri[:, c, 0:1])
            # transpose to O^T [ (c d), i ] and stage the a2a shard
            otb = sm_pool.tile([P, P], bf16, tag="otb", name=f"otb{pp}")
            nc.scalar.dma_start_transpose(
                otb[:], obq.rearrange("p a b -> p (a b)"))
            for c in range(4):
                nc.sync.dma_start(out=agi[pp][:, c * P:(c + 1) * P],
                                  in_=otb[c * D:(c + 1) * D, :])

        for p in range(NPASS + 1):
            if p < NPASS:
                ptqs[p % 2] = pt_pool.tile([P, N_SC_J, W], bf16, tag="pt",
                                           name=f"ptq{p}")
                ptqis[p % 2] = pt_pool.tile([P, N_DV_J, W], bf16, tag="pti",
                                            name=f"ptqi{p}")
            isl = slice((p % NPASS) * W, (p % NPASS) * W + W)
            def emit_scores(s):
                ptq, ptqi = ptqs[p % 2], ptqis[p % 2]
                st = st_pool.tile([P, 2, W], f32, tag="st",
                                  name=f"st{p}_{s}")
                for r in range(2):
                    j = 2 * s + r
                    b = 2 * (s % 2) + r   # row band: 4-way concurrency
                    nc.tensor.matmul(
                        st[:, r, :],
                        lhsT=kT4[32 * b:32 * (b + 1), j * P:(j + 1) * P],
                        rhs=qT4[32 * b:32 * (b + 1), isl],
                        start=True, stop=True,
                        tile_position=(32 * b, 0))
                return st

            def emit_exp(s, st):
                ptq, ptqi = ptqs[p % 2], ptqis[p % 2]
                if EXP_DVE[s]:
                    ro = _dv_rows[s]
                    nc.vector.tensor_scalar(
                        ptqi[:, ro:ro + 2, :].bitcast(i16), st[:],
                        scalar1=float(FE_A), scalar2=float(FE_B),
                        op0=ALU.mult, op1=ALU.add)
                else:
                    ro = _sc_rows[s]
                    nc.scalar.activation(ptq[:, ro:ro + 2, :], st[:],
                                         AF.Exp, scale=float(SCALE))

            def emit_av(s):
                pptq, pptqi = ptqs[(p - 1) % 2], ptqis[(p - 1) % 2]
                a = acc[0]
                rhs2 = []
                for r in range(2):
                    if EXP_DVE[s]:
                        rhs2.append(pptqi[:, _dv_rows[s] + r, :])
                    else:
                        rhs2.append(pptq[:, _sc_rows[s] + r, :])
                st_flag = dict(start=(s == 0), stop=(s == NSLOT - 1))
                nc.tensor.matmul(a[0:D, :], lhsT=vsb[:, 2 * s, :],
                                 rhs=rhs2[0], tile_position=(0, 0),
                                 **st_flag)
                nc.tensor.matmul(a[D:2 * D, :], lhsT=vsb[:, 2 * s + 1, :],
                                 rhs=rhs2[1], tile_position=(0, 32),
                                 **st_flag)
                nc.tensor.matmul(a[64:65, :], lhsT=ones1[:],
                                 rhs=rhs2[0], tile_position=(0, 64),
                                 **st_flag)
                nc.tensor.matmul(a[96:97, :], lhsT=ones1[:],
                                 rhs=rhs2[1], tile_position=(0, 96),
                                 **st_flag)

            if p > 0:
                acc[0] = acc_pool.tile([P, W], f32, tag="acc",
                                       name=f"acc{p - 1}")
            for u in range(NSLOT // 2):
                s0, s1 = 2 * u, 2 * u + 1
                if p < NPASS:
                    sta = emit_scores(s0)
                    stb = emit_scores(s1)
                    emit_exp(s0, sta)
                    emit_exp(s1, stb)
                if p > 0:
                    emit_av(s0)
                    emit_av(s1)
            if p > 0:
                emit_epilogue(p - 1)

        # ---------------- exchange + output projection ----------------
        nc.gpsimd.collective_compute(
            "AllToAll", ALU.bypass,
            replica_groups=[list(range(NCORES))],
            ins=[agi[:].opt()], outs=[ago[:].opt()])
        ago_flat = ago.rearrange("c d r -> (c d) r")
        osl = singles.tile([P, 2, W], bf16)
        for c in range(2):
            nc.sync.dma_start(out=osl[:, c, :],
                              in_=ago_flat[c * P:(c + 1) * P, :])
        yt = prj_pool.tile([P, 2, W], f32, tag="prj")
        for dc in range(2):
            for cin in range(2):
                nc.tensor.matmul(yt[:, dc, :],
                                 lhsT=wos[:, cin, dc * P:(dc + 1) * P],
                                 rhs=osl[:, cin, :],
                                 start=(cin == 0), stop=False)
            nc.tensor.matmul(yt[:, dc, :],
                             lhsT=bos[0:1, dc * P:(dc + 1) * P],
                             rhs=onesw[:], start=False, stop=True)
        yts = singles.tile([P, 2, W], f32)
        nc.vector.tensor_copy(yts[:, 0, :], yt[:, 0, :])
        nc.scalar.activation(yts[:, 1, :], yt[:, 1, :], AF.Copy)
        for dc in range(2):
            nc.sync.dma_start(out=outT[dc * P:(dc + 1) * P, :],
                              in_=yts[:, dc, :])

    nc.compile()
    return nc


def _get_built():
    global _BUILT
    if _BUILT is None:
        _BUILT = _build()
    return _BUILT


def make_in_maps(x, w_qkv, b_qkv, w_out, b_out):
    import ml_dtypes
    bf = ml_dtypes.bfloat16

    x = np.asarray(x, dtype=np.float32)
    w_qkv = np.asarray(w_qkv, dtype=np.float32)
    b_qkv = np.asarray(b_qkv, dtype=np.float32)
    w_out = np.asarray(w_out, dtype=np.float32)
    b_out = np.asarray(b_out, dtype=np.float32)

    xT = np.ascontiguousarray(x.T).astype(bf)
    wq3 = w_qkv.reshape(DIM, 3, H, D)       # [in, (q|k|v), head, d]
    bq3 = b_qkv.reshape(3, H, D)
    wob = np.ascontiguousarray(w_out).astype(bf)
    bob = np.ascontiguousarray(b_out.reshape(1, DIM)).astype(bf)
    in_maps = []
    for h in range(NCORES):
        # bq4[32r+d, g] = b_qkv[gate g, head h, dim d], band-replicated
        bq4 = np.ascontiguousarray(
            np.tile(bq3[:, h, :], (1, 4)).reshape(3, P).T).astype(np.float32)
        in_maps.append({
            "xb": xT,
            "wqkv": np.ascontiguousarray(
                wq3[:, :, h, :].reshape(DIM, 3 * D)).astype(bf),
            "bqkv4": bq4,
            "wo": wob,
            "bo": bob,
        })
    return in_maps


def assemble_output(results):
    # core h returns y^T [256, 512] for rows [512h, 512h+512)
    return np.concatenate(
        [np.ascontiguousarray(results[i]["outT"]).T.astype(np.float32)
         for i in range(NCORES)], axis=0)


def kernel(x, w_qkv, b_qkv, w_out, b_out):
    from concourse.bass_utils import run_bass_kernel_spmd

    nc = _get_built()
    in_maps = make_in_maps(x, w_qkv, b_qkv, w_out, b_out)
    res = run_bass_kernel_spmd(nc, in_maps, core_ids=list(range(NCORES)))
    return assemble_output(res.results)
